# revision 1
# baseline (speedup 1.0000x reference)
"""Trainium2 Bass kernel for nn_MultiHeadAttention_910533067646.

Self-contained: builds the Bass module, shards the full inputs across the
8 NeuronCores (data-parallel over batch x tensor-parallel over heads), runs
via PJRT, and reassembles the full output.

The reference module applies one shared projection p = x @ Wv.T + bv for
q=k=v, per-head softmax(p ph.T/8) @ ph, then a head-major (bugged) reshape
and output projection. The bugged reshape maps each head's attention output
to a disjoint 128-row block of the final output, so no cross-device
reduction is needed: device (b, hg) computes output rows
[1024*hg, 1024*hg+1024) of batch b.
"""
import numpy as np

from collections import deque
from contextlib import ExitStack

import concourse.bass as bass
import concourse.mybir as mybir
import concourse.tile as tile
from concourse.masks import make_identity

FP = mybir.dt.float32
FPR = mybir.dt.float32r
FP16 = mybir.dt.float16
BF16 = mybir.dt.bfloat16
Exp = mybir.ActivationFunctionType.Exp
ADD = mybir.AluOpType.add
MULT = mybir.AluOpType.mult


def _build_mha_nc(S=2048, D=1024, HL=8, dk=64, phases="ABCNF", MM=FPR,
                 loop_bcnf=1, dbg=False):
    EL = HL * dk            # local width of the value projection
    KK = D // 128           # contraction k-tiles
    NG = HL // 2            # head pairs
    NB = S // 128           # 128-row blocks of the sequence
    NBH = NB // 2           # blocks per sq-half
    SQH = S // 2            # sq-half width
    TT = D // dk            # total heads (= reshape block count)
    W = min(512, SQH)       # N-slice width for panels
    NSL = SQH // W
    WS = min(512, S)        # N-slice for pT phase
    NSS = S // WS
    WD = min(512, D)        # N-slice over D (output projection)
    NSD = D // WD
    assert EL <= 512 and SQH == D and S == 128 * TT and TT % 2 == 0

    nc = bass.Bass("TRN2")
    xT_d = nc.dram_tensor("xT", [D, S], FP, kind="ExternalInput")
    wvT_d = nc.dram_tensor("wvT", [D, EL], FP, kind="ExternalInput")
    woT_d = nc.dram_tensor("woT", [D, D], FP, kind="ExternalInput")
    bv_d = nc.dram_tensor("bv", [1, EL], FP, kind="ExternalInput")
    bo_d = nc.dram_tensor("bo", [1, D], FP, kind="ExternalInput")
    sel_d = nc.dram_tensor("sel", [2, 128], FP, kind="ExternalInput")
    if dbg:
        dbg_pT = nc.dram_tensor("dbg_pT", [128, NG * S], FP, kind="ExternalOutput")
        dbg_p = nc.dram_tensor("dbg_p", [128, NB * EL], FP16, kind="ExternalOutput")
        dbg_sums = nc.dram_tensor("dbg_sums", [128, 2 * NB * 2], FP,
                                  kind="ExternalOutput")
        dbg_recipT = nc.dram_tensor("dbg_recipT", [NB, 2 * 128], FP,
                                    kind="ExternalOutput")
        dbg_norm = nc.dram_tensor("dbg_norm", [128, S], FP, kind="ExternalOutput")
        dbg_rows = nc.dram_tensor("dbg_rows", [2, 2 * SQH], FP, kind="ExternalOutput")
        dbg_bc = nc.dram_tensor("dbg_bc", [128, 2 * SQH], FP, kind="ExternalOutput")
    out_d = nc.dram_tensor("out", [128 * HL, D], FP, kind="ExternalOutput")

    with ExitStack() as stk:
        tc = stk.enter_context(tile.TileContext(nc))
        const = stk.enter_context(tc.tile_pool(name="const", bufs=1))
        ppool = stk.enter_context(tc.tile_pool(name="ppool", bufs=1))
        epool = stk.enter_context(tc.tile_pool(name="epool", bufs=10))
        ps_m = stk.enter_context(tc.tile_pool(name="ps_m", bufs=3, space="PSUM"))

        bv_sb = const.tile([1, EL], MM, name="bv_sb")
        bo_sb = const.tile([1, D], MM, name="bo_sb")
        ones32 = const.tile([1, 512], FP, name="ones32")
        ones_sb = const.tile([1, 512], MM, name="ones_sb")
        sel_sb = const.tile([2, 128], MM, name="sel_sb")
        ones_bf = const.tile([1, 128], BF16, name="ones_bf")
        ident = const.tile([128, 128], FP, name="ident")
        bias_sb = const.tile([128, HL], FP, name="bias_sb")
        nc.sync.dma_start(bv_sb[:], bv_d[:].bitcast(MM))
        nc.sync.dma_start(bo_sb[:], bo_d[:].bitcast(MM))
        nc.gpsimd.memset(ones32[:], 1.0)
        nc.vector.tensor_copy(ones_sb[:], ones32[:])
        nc.vector.tensor_copy(ones_bf[:], ones32[0:1, 0:128])
        nc.sync.dma_start(sel_sb[:], sel_d[:].bitcast(MM))
        make_identity(nc, ident[:])

        pT_sb = ppool.tile([128, NG, S], MM, name="pT_sb")
        p_sb = ppool.tile([128, NB, EL], FP16, name="p_sb")

        xt_ctx = tc.tile_pool(name="xtpool", bufs=1)
        xtpool = xt_ctx.__enter__()
        wvT_sb = xtpool.tile([128, KK, EL], MM, name="wvT_sb")
        xT_sb = xtpool.tile([128, KK, S], MM, name="xT_sb")
        nc.sync.dma_start(wvT_sb[:],
                          wvT_d[:].bitcast(MM).rearrange("(kk p) e -> p kk e", p=128))
        for kk in range(KK):
            nc.sync.dma_start(xT_sb[:, kk, :],
                              xT_d[128 * kk:128 * (kk + 1), :].bitcast(MM))

        # ---- projection work units (phase A), emitted interleaved ----
        def emit_pT(g, ns):
            ps = ps_m.tile([128, WS], FP, name="ps_pt", tag="scores")
            for kk in range(KK):
                nc.tensor.matmul(ps[:], wvT_sb[:, kk, 128 * g:128 * (g + 1)],
                                 xT_sb[:, kk, WS * ns:WS * (ns + 1)],
                                 start=(kk == 0), stop=False)
            nc.tensor.matmul(ps[:], bv_sb[0:1, 128 * g:128 * (g + 1)],
                             ones_sb[0:1, 0:WS], start=False, stop=True)
            nc.vector.tensor_copy(pT_sb[:, g, WS * ns:WS * (ns + 1)], ps[:])

        def emit_p(j):
            ps = ps_m.tile([128, EL], FP, name="ps_p", tag="scores")
            for kk in range(KK):
                nc.tensor.matmul(ps[:], xT_sb[:, kk, 128 * j:128 * (j + 1)],
                                 wvT_sb[:, kk, :], start=(kk == 0), stop=False)
            nc.tensor.matmul(ps[:], ones_sb[0:1, 0:128], bv_sb[0:1, :],
                             start=False, stop=True)
            nc.vector.tensor_copy(p_sb[:, j, :], ps[:])

        proj_q = deque()
        for g in range(1, NG):
            for ns in range(NSS):
                proj_q.append(("pT", g, ns))

        def emit_proj(n):
            while n > 0 and proj_q:
                u = proj_q.popleft()
                if u[0] == "p":
                    emit_p(u[1])
                else:
                    emit_pT(u[1], u[2])
                n -= 1

        # prefix: pT for head-pair 0 and all of p (B/C/shift need them)
        for ns in range(NSS):
            emit_pT(0, ns)
        for j in range(NB):
            emit_p(j)

        post_pools = {}

        def ensure_post_pools():
            # opened once phase A is fully emitted: reuses xT address space
            if post_pools:
                return
            xt_ctx.__exit__(None, None, None)
            post_pools["w"] = stk.enter_context(tc.tile_pool(name="wpool", bufs=1))
            post_pools["n"] = stk.enter_context(tc.tile_pool(name="npool", bufs=2))
            post_pools["b"] = stk.enter_context(tc.tile_pool(name="bpool", bufs=2))
            post_pools["f"] = stk.enter_context(tc.tile_pool(name="fpool", bufs=2))
            post_pools["r"] = stk.enter_context(tc.tile_pool(name="rpool", bufs=2))
            woT_dup = post_pools["w"].tile([128, TT, D], MM, name="woT_dup")
            src = woT_d[:].bitcast(MM).rearrange("(t p) e -> p t e", p=dk)
            nc.sync.dma_start(woT_dup[0:dk, :, :], src)
            nc.sync.dma_start(woT_dup[dk:2 * dk, :, :], src)
            post_pools["woT"] = woT_dup

        if "B" not in phases:
            emit_proj(len(proj_q))
            ensure_post_pools()
        def emit_shift_all():
            # bias[:, al] = 10 - max_k(|p_k,al|^2)/8 for every local head al.
            # Cauchy-Schwarz: s[k,q] <= max_diag, so exp(s/8 + bias) <= e^10
            # keeps fp16 E from overflowing.
            dg = epool.tile([128, HL, NB], FP, name="dg", tag="dg", bufs=1)
            for j in range(NB):
                scr = epool.tile([128, HL, dk], FP, name="scr", tag="scr", bufs=2)
                nc.vector.tensor_tensor(
                    scr[:].rearrange("p a b -> p (a b)"), p_sb[:, j, :],
                    p_sb[:, j, :], MULT)
                nc.vector.reduce_sum(dg[:, :, j], scr[:],
                                     axis=mybir.AxisListType.X)
            cm8 = epool.tile([128, HL], FP, name="cm8", tag="cm8", bufs=1)
            nc.vector.reduce_max(cm8[:], dg[:], axis=mybir.AxisListType.X)
            ps_t1 = ps_m.tile([HL, 128], FP, name="ps_t1", tag="scores")
            nc.tensor.transpose(ps_t1[:], cm8[:], ident[:])
            cmT = epool.tile([HL, 128], FP, name="cmT", tag="cmT", bufs=1)
            nc.vector.tensor_copy(cmT[:], ps_t1[:])
            m8 = epool.tile([HL, 1], FP, name="m8", tag="m8", bufs=1)
            nc.vector.reduce_max(m8[:], cmT[:], axis=mybir.AxisListType.X)
            ps_t2 = ps_m.tile([1, HL], FP, name="ps_t2", tag="scores")
            nc.tensor.transpose(ps_t2[:], m8[:], ident[0:HL, 0:HL])
            m8T = epool.tile([1, HL], BF16, name="m8T", tag="m8T", bufs=1)
            nc.vector.tensor_copy(m8T[:], ps_t2[:])
            for al in range(HL):
                ps_b = ps_m.tile([128, 1], FP, name="ps_b", tag="scores")
                nc.tensor.matmul(ps_b[:], ones_bf[0:1, :], m8T[0:1, al:al + 1])
                nc.vector.tensor_scalar(bias_sb[:, al:al + 1], ps_b[:],
                                        -0.125, 10.0, MULT, ADD)

        loop_cm = None
        if loop_bcnf > 1:
            emit_proj(len(proj_q))
            ensure_post_pools()
            loop_cm = tc.For_i(0, loop_bcnf, 1)
            loop_cm.__enter__()
        shift_done = [False]
        pending_nf = [None]
        for g in range(NG if "B" in phases else 0):
            if not shift_done[0]:
                emit_shift_all()
                shift_done[0] = True
            sums = epool.tile([128, 2, NB, 2], FP, name="sums", tag="sums", bufs=2)
            outT_sb_box = [None]
            cpart = [None, None]

            W16 = min(512, SQH)
            NS16 = SQH // W16

            IH = NB // 2

            def emit_C_one(h, i, ns, a2, E):
                al = 2 * g + a2
                if cpart[h] is None:
                    cpart[h] = ps_m.tile([128, SQH], FP, name="cp",
                                         tag="cpart", bufs=1)
                nc.tensor.matmul(
                    cpart[h][64 * a2:64 * (a2 + 1), W16 * ns:W16 * (ns + 1)],
                    p_sb[:, i, dk * al:dk * (al + 1)],
                    E[:, W16 * ns:W16 * (ns + 1)],
                    tile_position=(0, 64 * a2),
                    start=(i % IH == 0), stop=(i % IH == IH - 1),
                    skip_group_check=True)

            def drain_C(h, first):
                if outT_sb_box[0] is None:
                    outT_sb_box[0] = post_pools["n"].tile(
                        [128, 2, SQH], FP, name="outT_sb", tag="outT_sb", bufs=2)
                outT_sb = outT_sb_box[0]
                if first:
                    nc.vector.tensor_copy(outT_sb[:, h, :], cpart[h][:])
                else:
                    nc.vector.tensor_tensor(outT_sb[:, h, :], cpart[h][:],
                                            outT_sb[:, h, :], ADD)
                cpart[h] = None

            for h in range(2):
                prev = None
                for i in range(NB):
                    emit_proj(2)
                    if not proj_q and not post_pools:
                        ensure_post_pools()
                    if h == 0 and i == 2 and pending_nf[0] is not None:
                        pending_nf[0]()
                        pending_nf[0] = None
                    cur = []
                    for a2 in range(2):
                        al_ = 2 * g + a2
                        lo, hi = 64 * a2, 64 * (a2 + 1)
                        # C matmuls of this head for step i-1 first: they
                        # depend only on this head's E(i-1), so they fill
                        # the PE gap while the other head's exp still runs
                        if prev is not None and "C" in phases:
                            for k in range(NS16):
                                ns = (k + a2) % NS16
                                emit_C_one(h, i - 1, ns, a2, prev[a2])
                            if a2 == 1 and i == IH:
                                drain_C(h, first=True)
                        sc = ps_m.tile([128, SQH], FP, name="sc", tag="scores")
                        for ns in range(NSL):
                            nc.tensor.matmul(
                                sc[:, W * ns:W * (ns + 1)],
                                pT_sb[lo:hi, g, 128 * i:128 * (i + 1)],
                                pT_sb[lo:hi, g,
                                      SQH * h + W * ns:SQH * h + W * (ns + 1)],
                                tile_position=(64 * a2, 0))
                        E = epool.tile([128, SQH], FP16, name="E", tag="E")
                        nc.scalar.activation(E[:], sc[:], Exp, scale=1.0 / 8.0,
                                             bias=bias_sb[:, al_:al_ + 1],
                                             accum_out=sums[:, a2, i, h:h + 1])
                        cur.append(E)
                    prev = cur
                if "C" in phases:
                    for k in range(NS16):
                        for a2 in range(2):
                            emit_C_one(h, NB - 1, (k + a2) % NS16, a2, prev[a2])
                    drain_C(h, first=False)

            emit_proj(len(proj_q))  # flush any phase-A leftovers
            ensure_post_pools()
            woT_dup = post_pools["woT"]
            if "N" not in phases:
                continue

            # ---- normalization ----

            def make_nf(g=g, sums=sums, outT_sb_box=outT_sb_box):
                def nf():
                    if dbg and g == 0:
                        nc.sync.dma_start(dbg_pT[:].bitcast(MM),
                                          pT_sb[:].rearrange("p a b -> p (a b)"))
                        nc.sync.dma_start(dbg_p[:], p_sb[:].rearrange("p a b -> p (a b)"))
                        nc.sync.dma_start(dbg_sums[:],
                                          sums[:].rearrange("p a b c -> p (a b c)"))
                    tot = epool.tile([128, 2, NB], FP, name="tot", tag="tot", bufs=2)
                    recipT = post_pools["r"].tile([NB, 2, 128], FP, name="recipT",
                                                  tag="recipT")
                    for a2 in range(2):
                        nc.vector.tensor_tensor(tot[:, a2, :], sums[:, a2, :, 0],
                                                sums[:, a2, :, 1], ADD)
                        nc.vector.reciprocal(tot[:, a2, :], tot[:, a2, :])
                        ps_t = ps_m.tile([NB, 128], FP, name="ps_t", tag="scores")
                        nc.tensor.transpose(ps_t[:], tot[:, a2, :], ident[:])
                        nc.vector.tensor_copy(recipT[:, a2, :], ps_t[:])
                    if dbg and g == 0:
                        nc.sync.dma_start(dbg_recipT[:],
                                          recipT[:].rearrange("p a b -> p (a b)"))
                    norm_g = post_pools["n"].tile([128, S], MM, name="norm_g", tag="nr")
                    for h in range(2):
                        rows2 = post_pools["r"].tile([2, SQH], MM, name="rows2",
                                                     tag="rows", bufs=2)
                        for a2 in range(2):
                            nc.sync.dma_start(
                                rows2[a2:a2 + 1, :],
                                recipT[NBH * h:NBH * (h + 1), a2, :].bitcast(MM))
                        # bc[p, n] = rows2[0, n] for p<64 else rows2[1, n] (K=2 matmul)
                        bc_ps = ps_m.tile([128, SQH], FP, name="bc_ps", tag="scores")
                        for ns in range(NSL):
                            nc.tensor.matmul(bc_ps[:, W * ns:W * (ns + 1)], sel_sb[:],
                                             rows2[:, W * ns:W * (ns + 1)])
                        bc = post_pools["b"].tile([128, SQH], FP, name="bc", tag="bc")
                        nc.vector.tensor_copy(bc[:], bc_ps[:])
                        if dbg and g == 0:
                            nc.sync.dma_start(dbg_rows[:, SQH * h:SQH * (h + 1)].bitcast(MM),
                                              rows2[:])
                            nc.sync.dma_start(dbg_bc[:, SQH * h:SQH * (h + 1)], bc[:])
                        nc.vector.tensor_tensor(norm_g[:, SQH * h:SQH * (h + 1)],
                                                outT_sb_box[0][:, h, :], bc[:], MULT)

                    if "F" not in phases:
                        return
                    if dbg and g == 0:
                        nc.sync.dma_start(dbg_norm[:], norm_g[:].bitcast(FP))
                    # ---- output projection (4-quadrant: a2 on rows, h on cols) ----
                    fps = [ps_m.tile([128, D], FP, name="fp_a", tag="scores"),
                           ps_m.tile([128, D], FP, name="fp_b", tag="cpart", bufs=1)]
                    for a2 in range(2):
                        for ns in range(NSD):
                            nc.tensor.matmul(fps[a2][:, WD * ns:WD * (ns + 1)],
                                             ones_sb[0:1, 0:128],
                                             bo_sb[0:1, WD * ns:WD * (ns + 1)],
                                             start=True, stop=False, skip_group_check=True)
                    for ns in range(NSD):
                        for t in range(TT):
                            for a2 in range(2):
                                lo = 64 * a2
                                nc.tensor.matmul(
                                    fps[a2][:, WD * ns:WD * (ns + 1)],
                                    norm_g[lo:lo + 64, t::TT],
                                    woT_dup[lo:lo + 64, t, WD * ns:WD * (ns + 1)],
                                    tile_position=(lo, 0),
                                    start=False, stop=(t == TT - 1),
                                    skip_group_check=True)
                    for a2 in range(2):
                        fsb = post_pools["f"].tile([128, D], FP, name="fsb", tag="fsb")
                        nc.vector.tensor_copy(fsb[:], fps[a2][:])
                        al = 2 * g + a2
                        nc.sync.dma_start(out_d[128 * al:128 * (al + 1), :], fsb[:])

                return nf

            if "N" in phases:
                make_nf()()

        if pending_nf[0] is not None:
            pending_nf[0]()
            pending_nf[0] = None
        if loop_cm is not None:
            loop_cm.__exit__(None, None, None)

    return nc


def _split_excess_waits(nc, max_waits=1):
    """This toolchain's walrus accepts only one sync-wait per instruction;
    hoist extra waits onto NoOps inserted just before."""
    fn = nc.m.functions[0]
    n_new = 0
    for blk in fn.blocks:
        new_insts = []
        for inst in blk.instructions:
            si = getattr(inst, 'sync_info', None)
            if si is not None and si.on_wait is not None \
                    and len(si.on_wait) > max_waits:
                waits = list(si.on_wait)
                while len(waits) > max_waits:
                    chunk, waits = waits[:max_waits], waits[max_waits:]
                    n_new += 1
                    new_insts.append(mybir.InstNoOp(
                        name=f"I-waitsplit-{n_new}", engine=inst.engine,
                        ins=[], outs=[],
                        sync_info=mybir.SyncInfo(on_wait=chunk, on_update=[]),
                        bass_nofuse=True))
                si.on_wait = waits
            new_insts.append(inst)
        blk.instructions = new_insts
    return n_new


class _PjrtRunner:
    def __init__(self, nc, n_cores):
        import jax
        from jax.sharding import Mesh, PartitionSpec
        from jax.experimental.shard_map import shard_map
        from concourse.bass2jax import (_bass_exec_p, partition_id_tensor,
                                        install_neuronx_cc_hook)
        install_neuronx_cc_hook()
        self.jax = jax
        self.n_cores = n_cores
        pname = nc.partition_id_tensor.name if nc.partition_id_tensor else None
        in_names, out_names, out_avals, zero_outs = [], [], [], []
        for alloc in nc.m.functions[0].allocations:
            if not isinstance(alloc, mybir.MemoryLocationSet):
                continue
            name = alloc.memorylocations[0].name
            if alloc.kind == "ExternalInput":
                if name != pname:
                    in_names.append(name)
            elif alloc.kind == "ExternalOutput":
                shape = tuple(alloc.tensor_shape)
                dtype = mybir.dt.np(alloc.dtype)
                out_names.append(name)
                out_avals.append(jax.core.ShapedArray(shape, dtype))
                zero_outs.append(np.zeros(shape, dtype))
        self.in_names, self.out_names = in_names, out_names
        self.out_avals, self.zero_outs = out_avals, zero_outs
        n_params, n_outs = len(in_names), len(out_avals)
        self.n_params = n_params
        all_in = in_names + out_names + ([pname] if pname else [])

        def _body(*args):
            operands = list(args)
            if pname is not None:
                operands.append(partition_id_tensor())
            return tuple(_bass_exec_p.bind(
                *operands, out_avals=tuple(out_avals), in_names=tuple(all_in),
                out_names=tuple(out_names), lowering_input_output_aliases=(),
                sim_require_finite=True, sim_require_nnan=True, nc=nc))

        devices = jax.devices()[:n_cores]
        self.mesh = Mesh(np.asarray(devices), ("core",))
        in_specs = (PartitionSpec("core"),) * (n_params + n_outs)
        out_specs = (PartitionSpec("core"),) * n_outs
        self.fn = jax.jit(
            shard_map(_body, mesh=self.mesh, in_specs=in_specs,
                      out_specs=out_specs, check_rep=False), keep_unused=True)
        self.PartitionSpec = PartitionSpec

    def run(self, in_maps):
        jax = self.jax
        per_core = [[np.asarray(m[n]) for n in self.in_names] for m in in_maps]
        concat_in = [np.concatenate([per_core[c][i] for c in range(self.n_cores)],
                                    axis=0) for i in range(self.n_params)]
        concat_zeros = [np.zeros((self.n_cores * z.shape[0], *z.shape[1:]),
                                 z.dtype) for z in self.zero_outs]
        sharding = jax.sharding.NamedSharding(self.mesh, self.PartitionSpec("core"))
        dev_in = [jax.device_put(a, sharding) for a in concat_in + concat_zeros]
        outs = self.fn(*dev_in)
        jax.block_until_ready(outs)
        return [
            {n: np.asarray(outs[i]).reshape(self.n_cores,
                                            *self.out_avals[i].shape)[c]
             for i, n in enumerate(self.out_names)}
            for c in range(self.n_cores)
        ]


_CACHE = {}

B_, S_, D_, H_, DK_ = 4, 2048, 1024, 16, 64
HL_ = H_ // 2          # heads per device
EL_ = HL_ * DK_        # value-projection width per device
_SEL = np.kron(np.eye(2), np.ones((1, 64))).astype(np.float32)


def kernel(x, Wv, bv, Wo, bo):
    x, Wv, bv = np.asarray(x), np.asarray(Wv), np.asarray(bv)
    Wo, bo = np.asarray(Wo), np.asarray(bo)
    if "r" not in _CACHE:
        nc = _build_mha_nc(S=S_, D=D_, HL=HL_, dk=DK_)
        _split_excess_waits(nc)
        _CACHE["r"] = _PjrtRunner(nc, 8)
    r = _CACHE["r"]
    woT = np.ascontiguousarray(Wo.T)
    in_maps = []
    for dev in range(8):
        b, hg = dev // 2, dev % 2
        in_maps.append({
            "xT": np.ascontiguousarray(x[b].T),
            "wvT": np.ascontiguousarray(Wv[EL_ * hg:EL_ * (hg + 1), :].T),
            "woT": woT,
            "bv": np.ascontiguousarray(bv[EL_ * hg:EL_ * (hg + 1)]).reshape(1, -1),
            "bo": np.ascontiguousarray(bo).reshape(1, -1),
            "sel": _SEL,
        })
    res = r.run(in_maps)
    out = np.zeros((B_, S_, D_), np.float32)
    for dev in range(8):
        b, hg = dev // 2, dev % 2
        out[b, 1024 * hg:1024 * (hg + 1), :] = res[dev]["out"]
    return out



# revision 38
# speedup vs baseline: 1.2743x; 1.2743x over previous
"""Trainium2 Bass kernel for nn_MultiHeadAttention_910533067646 (v2).

Self-contained: builds the Bass module, shards the full inputs across the
8 NeuronCores (data-parallel over batch x tensor-parallel over heads), runs
via PJRT, and reassembles the full output.

The reference module applies one shared projection p = x @ Wv.T + bv for
q=k=v, per-head softmax(p ph.T/8) @ ph, then a head-major (bugged) reshape
and output projection. The bugged reshape maps each head's attention output
to a disjoint 128-row block of the final output, so no cross-device
reduction is needed: device (b, hg) computes output rows
[1024*hg, 1024*hg+1024) of batch b, one 128-row block per local head.

v2 design (per core), all matmul operands fp16:
 - Phase A: chunked DMA of xT; p = x@WvT+bv once (PSUM ring as scratch);
   per-head-pair transposes of p build pT on the PE; global exp-overflow
   bias from max|p|^2.
 - Per head (8 sequential): for each 128-row query block i, one scores
   pass writes the full 2048-wide row into a 3-chunk PSUM ring (2 matmuls,
   K=64), one activation exps the pair of chunks [128,2048] with accum_out
   giving the complete softmax denominator, and 4 matmuls accumulate the
   attention output via E-symmetry into a parity-packed accumulator
   cpart[(jp,d), m] (q = 2m+jp). Normalization uses reciprocal + a
   partition_broadcast (Pool engine) of the parity-split recips; the output
   projection then runs at K=128 full rate with Wo pre-arranged on the host
   into the matching (jp,d)-partition layout.
"""
import numpy as np

from contextlib import ExitStack

import concourse.bass as bass
import concourse.mybir as mybir
import concourse.tile as tile
from concourse.masks import make_identity

FP = mybir.dt.float32
FPR = mybir.dt.float32r
FP16 = mybir.dt.float16
Exp = mybir.ActivationFunctionType.Exp
ADD = mybir.AluOpType.add
MULT = mybir.AluOpType.mult


def _build_mha_v2(S=2048, D=1024, HL=8, dk=64, phases="ABNF", loop_bcnf=1,
                  dbg=False):
    EL = HL * dk            # 512: local width of the value projection
    KK = D // 128           # 8 contraction k-tiles
    NB = S // 128           # 16 query blocks
    NG = HL // 2            # 4 head pairs
    S2 = S // 2             # 1024
    assert EL == 512 and S2 == 1024 and NB == 16

    nc = bass.Bass("TRN2")
    xT_d = nc.dram_tensor("xT", [D, S], FP16, kind="ExternalInput")
    wvT_d = nc.dram_tensor("wvT", [D, EL], FP16, kind="ExternalInput")
    woTP_d = nc.dram_tensor("woTP", [128, KK, D], FP16, kind="ExternalInput")
    bv_d = nc.dram_tensor("bv", [1, EL], FP16, kind="ExternalInput")
    bo_d = nc.dram_tensor("bo", [1, D], FP16, kind="ExternalInput")
    sel_d = nc.dram_tensor("sel", [2, 128], FP, kind="ExternalInput")
    out_d = nc.dram_tensor("out", [128 * HL, D], FP, kind="ExternalOutput")
    if dbg:
        dbg_p = nc.dram_tensor("dbg_p", [128, NB * EL], FP16, kind="ExternalOutput")
        dbg_pT = nc.dram_tensor("dbg_pT", [128, NG * S], FP16, kind="ExternalOutput")
        dbg_E = nc.dram_tensor("dbg_E", [128, S], FP16, kind="ExternalOutput")
        dbg_sums = nc.dram_tensor("dbg_sums", [128, NB], FP, kind="ExternalOutput")
        dbg_outP = nc.dram_tensor("dbg_outP", [128, S2], FP, kind="ExternalOutput")
        dbg_bc = nc.dram_tensor("dbg_bc", [128, S2], FP, kind="ExternalOutput")
        dbg_normP = nc.dram_tensor("dbg_normP", [128, S2], FP16,
                                   kind="ExternalOutput")
        dbg_bias = nc.dram_tensor("dbg_bias", [128, 1], FP, kind="ExternalOutput")

    with ExitStack() as stk:
        tc = stk.enter_context(tile.TileContext(nc))
        const = stk.enter_context(tc.tile_pool(name="const", bufs=1))
        main = stk.enter_context(tc.tile_pool(name="main", bufs=1))
        epool = stk.enter_context(tc.tile_pool(name="epool", bufs=8))
        spool = stk.enter_context(tc.tile_pool(name="spool", bufs=2))
        npool = stk.enter_context(tc.tile_pool(name="npool", bufs=2))
        ps_sc = stk.enter_context(tc.tile_pool(name="ps_sc", bufs=3,
                                               space="PSUM"))
        ps_acc = stk.enter_context(tc.tile_pool(name="ps_acc", bufs=1,
                                                space="PSUM"))

        # ---- constants ----
        ones32 = const.tile([1, 128], FP, name="ones32")
        ones16 = const.tile([1, 128], FP16, name="ones16")
        ident32 = const.tile([128, 128], FP, name="ident32")
        ident16 = const.tile([128, 128], FP16, name="ident16")
        sel_sb = const.tile([2, 128], FPR, name="sel_sb")
        bv_sb = const.tile([1, EL], FP16, name="bv_sb")
        bo_sb = const.tile([1, D], FP16, name="bo_sb")
        bias_g = const.tile([128, 1], FP, name="bias_g")
        nc.gpsimd.memset(ones32[:], 1.0)
        nc.vector.tensor_copy(ones16[:], ones32[:])
        make_identity(nc, ident32[:])
        nc.vector.tensor_copy(ident16[:], ident32[:])
        # sel[jp, jp*64+d] = 1: K=2 matmul broadcasts the two parity rows of
        # recips onto partition halves (jp, d)
        nc.sync.dma_start(sel_sb[:], sel_d[:].bitcast(FPR))
        nc.sync.dma_start(bv_sb[:], bv_d[:])
        nc.sync.dma_start(bo_sb[:], bo_d[:])

        # ---- persistent SBUF ----
        xT_sb = main.tile([128, KK, S], FP16, name="xT_sb")
        wvT_sb = main.tile([128, KK, EL], FP16, name="wvT_sb")
        woTP_sb = main.tile([128, KK, D], FP16, name="woTP_sb")
        p_sb = main.tile([128, NB, EL], FP16, name="p_sb")
        pT_sb = main.tile([128, NG, S], FP16, name="pT_sb")

        nc.sync.dma_start(wvT_sb[:],
                          wvT_d[:].rearrange("(kk p) e -> p kk e", p=128))
        # xT in 4 chunks of 4 query blocks each so projection can start early
        xT_r = xT_d[:].rearrange("(kk p) s -> p kk s", p=128)
        for ch in range(4):
            nc.sync.dma_start(xT_sb[:, :, 512 * ch:512 * (ch + 1)],
                              xT_r[:, :, 512 * ch:512 * (ch + 1)])
        # woTP is first needed by the output projection, ~40us in
        nc.sync.dma_start(woTP_sb[:], woTP_d[:])

        # ---- phase A: projection p, transposes pT, global bias ----
        # batched per-engine stages (pipelined by the in-order engine queues)
        # rather than a per-j cross-engine chain, which is sem-latency bound
        dg = spool.tile([128, HL, NB], FP, name="dg", tag="dg", bufs=1)
        sq_sb = main.tile([128, NB, EL], FP16, name="sq_sb")
        for j in range(NB):
            ps = ps_sc.tile([128, S2], FP, name="sc", tag="sc")[:, 0:512]
            for kk in range(KK):
                nc.tensor.matmul(ps, xT_sb[:, kk, 128 * j:128 * (j + 1)],
                                 wvT_sb[:, kk, :], start=(kk == 0), stop=False)
            nc.tensor.matmul(ps, ones16[0:1, :], bv_sb[0:1, :],
                             start=False, stop=True)
            nc.vector.tensor_copy(p_sb[:, j, :], ps)
            # |p|^2 on the otherwise-idle ACT engine
            nc.scalar.square(sq_sb[:, j, :], p_sb[:, j, :])
        for j in range(NB):
            nc.vector.reduce_sum(dg[:, :, j],
                                 sq_sb[:, j, :].rearrange("p (a b) -> p a b",
                                                          a=HL),
                                 axis=mybir.AxisListType.X)

        # global bias: 10 - max|p|^2 / 8 (Cauchy-Schwarz bound keeps E <= e^10)
        cm8 = spool.tile([128, HL], FP, name="cm8", tag="cm8", bufs=1)
        nc.vector.reduce_max(cm8[:], dg[:], axis=mybir.AxisListType.X)
        cm1 = spool.tile([128, 1], FP, name="cm1", tag="cm1", bufs=1)
        nc.vector.reduce_max(cm1[:], cm8[:], axis=mybir.AxisListType.X)
        acc = ps_acc.tile([128, S2], FP, name="acc_b", tag="acc")
        nc.tensor.transpose(acc[0:1, 0:128], cm1[:], ident32[:])
        cmT = spool.tile([1, 128], FP, name="cmT", tag="cmT", bufs=1)
        nc.vector.reduce_max(cmT[0:1, 0:1], acc[0:1, 0:128],
                             axis=mybir.AxisListType.X)
        ps_b = ps_sc.tile([128, S2], FP, name="sc", tag="sc")
        nc.tensor.matmul(ps_b[:, 0:1], ones32[0:1, :], cmT[0:1, 0:1])
        nc.vector.tensor_scalar(bias_g[:], ps_b[:, 0:1], -0.125, 10.0,
                                MULT, ADD)

        # pT via PE transposes, grouped per head-pair g: 16 transposes into
        # one PSUM tile, then a single wide fp16 copy out
        for g in range(NG):
            acc = ps_acc.tile([128, S2], FP, name="acc_t", tag="acc")
            accT = acc.bitcast(FP16)
            for j in range(NB):
                nc.tensor.transpose(accT[:, 128 * j:128 * (j + 1)],
                                    p_sb[:, j, 128 * g:128 * (g + 1)],
                                    ident16[:])
            nc.vector.tensor_copy(pT_sb[:, g, :], accT[:])
        if dbg:
            nc.sync.dma_start(dbg_bias[:], bias_g[:])
            nc.sync.dma_start(dbg_p[:], p_sb[:].rearrange("p a b -> p (a b)"))
            nc.sync.dma_start(dbg_pT[:], pT_sb[:].rearrange("p a b -> p (a b)"))

        if "B" not in phases:
            return nc

        # ---- steady state: 8 heads, B/N/F pipelined ----
        loop_cm = None
        if loop_bcnf > 1:
            loop_cm = tc.For_i(0, loop_bcnf, 1)
            loop_cm.__enter__()

        from collections import deque
        # aux PE work (previous head's N+F pieces), metered between steps so
        # queued PE work never starves the activation engine of scores
        aux_q = deque()            # (cost_ns, closure)
        AUX_BUDGET = 1500.0

        def emit_head(t):
            g, a2 = t // 2, t % 2
            lo = 64 * a2
            sums = spool.tile([128, NB], FP, name="sums", tag="sums")
            sums2 = spool.tile([128, NB], FP, name="sums2", tag="sums2")
            cpart_box = [None]
            Es = {}
            cq = deque()           # deferred C block ids of this head

            def emit_c(i):
                # acc slot claimed lazily: only after the previous head's fps
                # (same PSUM slot) has fully been emitted.
                if cpart_box[0] is None:
                    cpart_box[0] = ps_acc.tile([128, S2], FP, name="cpart",
                                               tag="acc")
                cpart = cpart_box[0]
                E = Es.pop(i)
                for jp in range(2):
                    # E columns are q-ordered; stride-2 picks parity jp
                    for ns in range(2):
                        nc.tensor.matmul(
                            cpart[64 * jp:64 * (jp + 1),
                                  512 * ns:512 * (ns + 1)],
                            p_sb[:, i, 64 * t:64 * (t + 1)],
                            E[:, S2 * ns + jp::2][:, 0:512],
                            tile_position=(0, 64 * jp),
                            start=(i == 0), stop=(i == NB - 1),
                            skip_group_check=True)

            for i in range(NB):
                E = epool.tile([128, S], FP16, name="E", tag="E")
                for h, acc_out in ((0, sums), (1, sums2)):
                    sc = ps_sc.tile([128, S2], FP, name="sc", tag="sc")
                    for ns in range(2):
                        nc.tensor.matmul(
                            sc[:, 512 * ns:512 * (ns + 1)],
                            pT_sb[lo:lo + 64, g, 128 * i:128 * (i + 1)],
                            pT_sb[lo:lo + 64, g,
                                  S2 * h + 512 * ns:S2 * h + 512 * (ns + 1)],
                            tile_position=(lo, 0))
                    nc.scalar.activation(
                        E[:, S2 * h:S2 * (h + 1)], sc[:],
                        Exp, scale=0.125, bias=bias_g[:, 0:1],
                        accum_out=acc_out[:, i:i + 1])
                if dbg and t == 0 and i == 0:
                    nc.sync.dma_start(dbg_E[:], E[:])
                Es[i] = E
                cq.append(i)
                budget = AUX_BUDGET
                while budget > 0:
                    if aux_q:
                        cost, fn = aux_q.popleft()
                        fn()
                        budget -= cost
                    elif cq and cq[0] <= i - 2:
                        # C lag >= 2: its exp is long done, so it never gates
                        # the following scores in the in-order PE queue
                        emit_c(cq.popleft())
                        budget -= 852
                    else:
                        break
            while cq:
                emit_c(cq.popleft())

            cpart = cpart_box[0]

            def nf0():
                recipS = spool.tile([128, NB], FP, name="recipS", tag="rS")
                nc.vector.tensor_tensor(sums[:], sums[:], sums2[:], ADD)
                nc.vector.reciprocal(recipS[:], sums[:])
                rT = ps_sc.tile([128, S2], FP, name="sc", tag="sc")[0:NB, 0:128]
                nc.tensor.transpose(rT, recipS[:], ident32[:])
                recipT = spool.tile([NB, 128], FP, name="recipT", tag="rT")
                nc.vector.tensor_copy(recipT[:], rT)
                rows2P = spool.tile([2, S2], FPR, name="rows2P", tag="r2")
                for jp in range(2):
                    nc.sync.dma_start(rows2P[jp:jp + 1, :],
                                      recipT[:, jp::2].bitcast(FPR))
                bc_ps = ps_sc.tile([128, S2], FP, name="sc", tag="sc")
                for ns in range(2):
                    nc.tensor.matmul(bc_ps[:, 512 * ns:512 * (ns + 1)],
                                     sel_sb[:],
                                     rows2P[:, 512 * ns:512 * (ns + 1)])
                bc = npool.tile([128, S2], FP, name="bc", tag="bc")
                nc.vector.tensor_copy(bc[:], bc_ps[:])
                outP = npool.tile([128, S2], FP, name="outP", tag="outP")
                nc.vector.tensor_copy(outP[:], cpart[:])
                normP = npool.tile([128, S2], FP16, name="normP", tag="nP")
                nc.vector.tensor_tensor(normP[:], outP[:], bc[:], MULT)
                if dbg and t == 0:
                    nc.sync.dma_start(dbg_sums[:], sums[:])
                    nc.sync.dma_start(dbg_outP[:], outP[:])
                    nc.sync.dma_start(dbg_bc[:], bc[:])
                    nc.sync.dma_start(dbg_normP[:], normP[:])
                st["normP"] = normP
                st["fps"] = ps_acc.tile([128, D], FP, name="fps", tag="acc")

            st = {}

            def f_piece(kks, with_bias):
                def fn():
                    for ns in range(2):
                        sl = slice(512 * ns, 512 * (ns + 1))
                        if with_bias:
                            nc.tensor.matmul(st["fps"][:, sl], ones16[0:1, :],
                                             bo_sb[0:1, sl], start=True,
                                             stop=False, skip_group_check=True)
                        for kk in kks:
                            nc.tensor.matmul(st["fps"][:, sl],
                                             st["normP"][:, kk::KK],
                                             woTP_sb[:, kk, sl],
                                             start=False,
                                             stop=(kk == KK - 1),
                                             skip_group_check=True)
                return fn

            def f_out():
                fsb = npool.tile([128, D], FP, name="fsb", tag="fsb")
                nc.vector.tensor_copy(fsb[:], st["fps"][:])
                nc.sync.dma_start(out_d[128 * t:128 * (t + 1), :], fsb[:])

            aux_q.append((400.0, nf0))
            aux_q.append((854.0, f_piece(range(0, 1), True)))
            aux_q.append((854.0, f_piece(range(1, 3), False)))
            aux_q.append((854.0, f_piece(range(3, 5), False)))
            aux_q.append((854.0, f_piece(range(5, 7), False)))
            aux_q.append((630.0, f_piece(range(7, 8), False)))
            aux_q.append((200.0, f_out))

        for t in range(HL):
            emit_head(t)
        # tail: flush the last head's N+F
        while aux_q:
            aux_q.popleft()[1]()

        if loop_cm is not None:
            loop_cm.__exit__(None, None, None)

    return nc


def _split_excess_waits(nc, max_waits=1):
    """This toolchain's walrus accepts only one sync-wait per instruction;
    hoist extra waits onto NoOps inserted just before."""
    fn = nc.m.functions[0]
    n_new = 0
    for blk in fn.blocks:
        new_insts = []
        for inst in blk.instructions:
            si = getattr(inst, 'sync_info', None)
            if si is not None and si.on_wait is not None \
                    and len(si.on_wait) > max_waits:
                waits = list(si.on_wait)
                while len(waits) > max_waits:
                    chunk, waits = waits[:max_waits], waits[max_waits:]
                    n_new += 1
                    new_insts.append(mybir.InstNoOp(
                        name=f"I-waitsplit-{n_new}", engine=inst.engine,
                        ins=[], outs=[],
                        sync_info=mybir.SyncInfo(on_wait=chunk, on_update=[]),
                        bass_nofuse=True))
                si.on_wait = waits
            new_insts.append(inst)
        blk.instructions = new_insts
    return n_new


class _PjrtRunner:
    def __init__(self, nc, n_cores):
        import jax
        from jax.sharding import Mesh, PartitionSpec
        from jax.experimental.shard_map import shard_map
        from concourse.bass2jax import (_bass_exec_p, partition_id_tensor,
                                        install_neuronx_cc_hook)
        install_neuronx_cc_hook()
        self.jax = jax
        self.n_cores = n_cores
        pname = nc.partition_id_tensor.name if nc.partition_id_tensor else None
        in_names, out_names, out_avals, zero_outs = [], [], [], []
        for alloc in nc.m.functions[0].allocations:
            if not isinstance(alloc, mybir.MemoryLocationSet):
                continue
            name = alloc.memorylocations[0].name
            if alloc.kind == "ExternalInput":
                if name != pname:
                    in_names.append(name)
            elif alloc.kind == "ExternalOutput":
                shape = tuple(alloc.tensor_shape)
                dtype = mybir.dt.np(alloc.dtype)
                out_names.append(name)
                out_avals.append(jax.core.ShapedArray(shape, dtype))
                zero_outs.append(np.zeros(shape, dtype))
        self.in_names, self.out_names = in_names, out_names
        self.out_avals, self.zero_outs = out_avals, zero_outs
        n_params, n_outs = len(in_names), len(out_avals)
        self.n_params = n_params
        all_in = in_names + out_names + ([pname] if pname else [])

        def _body(*args):
            operands = list(args)
            if pname is not None:
                operands.append(partition_id_tensor())
            return tuple(_bass_exec_p.bind(
                *operands, out_avals=tuple(out_avals), in_names=tuple(all_in),
                out_names=tuple(out_names), lowering_input_output_aliases=(),
                sim_require_finite=True, sim_require_nnan=True, nc=nc))

        devices = jax.devices()[:n_cores]
        self.mesh = Mesh(np.asarray(devices), ("core",))
        in_specs = (PartitionSpec("core"),) * (n_params + n_outs)
        out_specs = (PartitionSpec("core"),) * n_outs
        self.fn = jax.jit(
            shard_map(_body, mesh=self.mesh, in_specs=in_specs,
                      out_specs=out_specs, check_rep=False), keep_unused=True)
        self.PartitionSpec = PartitionSpec

    def run(self, in_maps):
        jax = self.jax
        per_core = [[np.asarray(m[n]) for n in self.in_names] for m in in_maps]
        concat_in = [np.concatenate([per_core[c][i] for c in range(self.n_cores)],
                                    axis=0) for i in range(self.n_params)]
        concat_zeros = [np.zeros((self.n_cores * z.shape[0], *z.shape[1:]),
                                 z.dtype) for z in self.zero_outs]
        sharding = jax.sharding.NamedSharding(self.mesh, self.PartitionSpec("core"))
        dev_in = [jax.device_put(a, sharding) for a in concat_in + concat_zeros]
        outs = self.fn(*dev_in)
        jax.block_until_ready(outs)
        return [
            {n: np.asarray(outs[i]).reshape(self.n_cores,
                                            *self.out_avals[i].shape)[c]
             for i, n in enumerate(self.out_names)}
            for c in range(self.n_cores)
        ]


_CACHE = {}

B_, S_, D_, H_, DK_ = 4, 2048, 1024, 16, 64
HL_ = H_ // 2          # heads per device
EL_ = HL_ * DK_        # value-projection width per device
KK_ = D_ // 128
_SEL = np.kron(np.eye(2), np.ones((1, 64))).astype(np.float32)


def _in_maps(x, Wv, bv, Wo, bo):
    """Per-core input dict list (host-side shard + fp16 convert + relayout)."""
    woT = np.ascontiguousarray(Wo.T)                      # [e', e] -> WoT[m, e]
    woTP = np.ascontiguousarray(
        woT.reshape(KK_, 2, 64, D_).transpose(1, 2, 0, 3)
        .reshape(128, KK_, D_)).astype(np.float16)
    maps = []
    for dev in range(8):
        b, hg = dev // 2, dev % 2
        maps.append({
            "xT": np.ascontiguousarray(x[b].T).astype(np.float16),
            "wvT": np.ascontiguousarray(
                Wv[EL_ * hg:EL_ * (hg + 1), :].T).astype(np.float16),
            "woTP": woTP,
            "bv": bv[EL_ * hg:EL_ * (hg + 1)].reshape(1, -1).astype(np.float16),
            "bo": bo.reshape(1, -1).astype(np.float16),
            "sel": _SEL,
        })
    return maps


def kernel(x, Wv, bv, Wo, bo):
    x, Wv, bv = np.asarray(x), np.asarray(Wv), np.asarray(bv)
    Wo, bo = np.asarray(Wo), np.asarray(bo)
    if "r" not in _CACHE:
        nc = _build_mha_v2()
        _split_excess_waits(nc)
        _CACHE["r"] = _PjrtRunner(nc, 8)
    r = _CACHE["r"]
    res = r.run(_in_maps(x, Wv, bv, Wo, bo))
    out = np.zeros((B_, S_, D_), np.float32)
    for dev in range(8):
        b, hg = dev // 2, dev % 2
        out[b, 1024 * hg:1024 * (hg + 1), :] = res[dev]["out"]
    return out
